# revision 17
# baseline (speedup 1.0000x reference)
"""DEMA (double exponential smoothing) Trainium2 kernel — fp16 I/O.

x: [64, 2048, 512] fp32; recurrence over T=2048 is a 2x2 linear
time-invariant system per (batch, channel) lane:

    z_t = A z_{t-1} + B x_t,   y_t = e1^T z_t
    A = [[1-a, 1-a], [-ab, 1-ab]],  B = [a, ab]^T

Blocked scan: chunks of L=126 timesteps. One [128x128] @ [128x512]
matmul per (batch, chunk): rhs rows 0-1 carry the (s, b) state into
the chunk, rows 2..127 carry the chunk's inputs; lhsT columns 0-1
produce the chunk-end state (fed into the next chunk's rhs rows 0-1
via a tiny PSUM->SBUF copy), columns 2..127 produce the outputs.
Batch dim is sharded 8 ways across cores (8 batches per core).

The kernel is HBM-bandwidth bound (~358 GB/s per core). The rel-err
budget (2e-2) dwarfs fp16 quantization (~5e-4 measured end-to-end),
so all HBM traffic is fp16: the host casts x to fp16 per shard, the
kernel computes fp16 matmuls with fp32 PSUM accumulation, writes the
output in fp16, and the host upcasts. That halves traffic vs fp32 to
33 MB/core (~92 us roofline).

DMA plan: ONE read and ONE write dma_start per round, each moving all
8 batches (~1 MB) via a 3D access pattern ([t, b, c]; contiguous 1 KB
runs >= the 512 B line-rate threshold). Reads ride the SP HWDGE ring,
writes the ACT ring — separate FIFOs, so a draining write never
head-of-line-blocks a read; the 16 SDMA engines round-robin between
the two rings at packet granularity, sharing HBM bandwidth.
"""

import sys

import numpy as np

if "/opt/trn_rl_repo" not in sys.path:
    sys.path.insert(0, "/opt/trn_rl_repo")

B, T, C = 64, 2048, 512
NCORES = 8
BPC = B // NCORES  # batches per core
L = 126            # timesteps per full chunk (126 outputs + 2 state rows = 128)
NFULL = 16         # full chunks cover t = 0..2015
LT = T - NFULL * L  # tail chunk, 32 timesteps

NG = 4             # batch groups per round (PSUM granularity)
GB = BPC // NG     # batches per group (2) -> one PSUM tile is [128, GB, 512]

_cache = {}


def _build_mats(alpha, beta):
    """Per-call host precompute of the chunk transfer matrices (float64)."""
    a = np.float64(alpha)
    b = np.float64(beta)
    A = np.array([[1 - a, 1 - a], [-a * b, 1 - a * b]], dtype=np.float64)
    Bv = np.array([a, a * b], dtype=np.float64)
    Ap = [np.eye(2)]
    for _ in range(L):
        Ap.append(Ap[-1] @ A)
    AB = np.stack([Ap[j] @ Bv for j in range(L)])  # [L, 2], A^j B
    w = AB[:, 0]                                   # w_j = e1^T A^j B

    # Generic chunk starting at t0, carry z_{t0-1} in rhs rows 0-1:
    #   z_{t0+tau} = A^{tau+1} z_{t0-1} + sum_k A^{tau-k} B x_{t0+k}
    G1 = np.zeros((128, 128))
    for tau in range(L):
        m = 2 + tau
        G1[0, m] = Ap[tau + 1][0, 0]
        G1[1, m] = Ap[tau + 1][0, 1]
        for k in range(tau + 1):
            G1[2 + k, m] = w[tau - k]
    for j in range(2):
        for jp in range(2):
            G1[j, jp] = Ap[L][jp, j]
    for k in range(L):
        G1[2 + k, 0] = AB[L - 1 - k][0]
        G1[2 + k, 1] = AB[L - 1 - k][1]

    # Chunk 0: z_0 = (x_0, x_1 - x_0), y_0 = x_0, rhs rows 0-1 are zero
    # (and dropped: G0 is [126, 128], round 0's rhs is pure inputs).
    G0 = np.zeros((128, 128))
    G0[2, 2] = 1.0
    for tau in range(1, L):
        m = 2 + tau
        G0[2, m] = Ap[tau][0, 0] - Ap[tau][0, 1]
        G0[3, m] = Ap[tau][0, 1] + w[tau - 1]
        for k in range(2, tau + 1):
            G0[2 + k, m] = w[tau - k]
    for jp in range(2):
        G0[2, jp] = Ap[L - 1][jp, 0] - Ap[L - 1][jp, 1]
        G0[3, jp] = Ap[L - 1][jp, 1] + AB[L - 2][jp]
        for k in range(2, L):
            G0[2 + k, jp] = AB[L - 1 - k][jp]

    # Tail chunk: LT outputs, no state columns.
    Gt = np.zeros((2 + LT, LT))
    for tau in range(LT):
        Gt[0, tau] = Ap[tau + 1][0, 0]
        Gt[1, tau] = Ap[tau + 1][0, 1]
        for k in range(tau + 1):
            Gt[2 + k, tau] = w[tau - k]
    return (
        G0[2:128].astype(np.float16),
        G1.astype(np.float16),
        Gt.astype(np.float16),
    )


def _build_program():
    import concourse.mybir as mybir
    import concourse.tile as tile
    from concourse import bacc

    FP16 = mybir.dt.float16
    FP32 = mybir.dt.float32
    nc = bacc.Bacc(
        "TRN2", target_bir_lowering=False, debug=False, enable_asserts=False
    )
    # x/y live in DRAM pre-permuted to [t, b, c] (host does the transpose):
    # each round's read/write is then one contiguous ~1 MB slab -> 126
    # descriptors of 8 KB instead of 1008 of 1 KB (HWDGE DIRECT2D issue
    # cost and SDMA per-descriptor overhead both drop ~8x).
    x_d = nc.dram_tensor("x", [T, BPC, C], FP16, kind="ExternalInput").ap()
    g0_d = nc.dram_tensor("g0", [L, 128], FP16, kind="ExternalInput").ap()
    g1_d = nc.dram_tensor("g1", [128, 128], FP16, kind="ExternalInput").ap()
    gt_d = nc.dram_tensor("gt", [2 + LT, LT], FP16, kind="ExternalInput").ap()
    y_d = nc.dram_tensor("y", [T, BPC, C], FP16, kind="ExternalOutput").ap()

    with tile.TileContext(nc) as tc:
        with (
            tc.tile_pool(name="g", bufs=1) as gpool,
            tc.tile_pool(name="xp", bufs=4) as xpool,
            tc.tile_pool(name="op", bufs=3) as opool,
            tc.tile_pool(name="ps", bufs=4, space="PSUM") as pspool,
        ):
            g0 = gpool.tile([L, 128], FP16, tag="g0")
            g1 = gpool.tile([128, 128], FP16, tag="g1")
            gt = gpool.tile([2 + LT, LT], FP16, tag="gt")
            nc.scalar.dma_start(out=g0[:], in_=g0_d)
            nc.scalar.dma_start(out=g1[:], in_=g1_d)
            nc.scalar.dma_start(out=gt[:], in_=gt_d)

            def read_round(j):
                """Allocate round j's input tile + issue its read DMA."""
                if j == 0:
                    t = xpool.tile([L, BPC, C], FP16, tag="x")
                    nc.sync.dma_start(out=t[:], in_=x_d[0:L, :, :])
                    return t
                nrows = L if j < NFULL else LT
                t = xpool.tile([2 + nrows, BPC, C], FP16, tag="x")
                nc.sync.dma_start(
                    out=t[2:2 + nrows, :, :],
                    in_=x_d[L * j:L * j + nrows, :, :],
                )
                return t

            # Reads run 2 rounds ahead so a ~6 us DMA completion latency
            # never paces the round loop.
            xt = [read_round(0), read_round(1)]
            oprev = None

            for i in range(NFULL + 1):
                xs = xt[i]
                if i + 2 <= NFULL:
                    xt.append(read_round(i + 2))
                # round i-1's write: issued on the sync ring right after
                # the prefetch so neither DIRECT2D sits in the scalar/
                # vector cast chain; o(i-1) is complete, so no sem stall.
                if i >= 1:
                    nc.sync.dma_start(
                        out=y_d[L * (i - 1):L * i, :, :],
                        in_=oprev[2:, :, :],
                    )
                tail = i == NFULL
                orows = LT if tail else 128
                o = opool.tile([orows, BPC, C], FP16, tag="o")
                for g in range(NG):
                    bsl = slice(g * GB, (g + 1) * GB)
                    ps = pspool.tile([orows, GB, C], FP32, tag="ps")
                    for bb in range(GB):
                        b = g * GB + bb
                        if tail:
                            nc.tensor.matmul(
                                ps[:, bb, :], gt[:], xs[0:2 + LT, b, :],
                                start=True, stop=True,
                            )
                        elif i == 0:
                            nc.tensor.matmul(
                                ps[:, bb, :], g0[:], xs[0:L, b, :],
                                start=True, stop=True,
                            )
                        else:
                            nc.tensor.matmul(
                                ps[:, bb, :], g1[:], xs[:, b, :],
                                start=True, stop=True,
                            )
                    # PSUM -> SBUF output copy (casts fp32 -> fp16); rows
                    # 0-1 get the chunk-end states for free. Alternate
                    # scalar/vector: 2 copies each (~2.2 us/round).
                    if g % 2 == 0:
                        nc.scalar.copy(out=o[:, bsl, :], in_=ps[:])
                    else:
                        nc.vector.tensor_copy(out=o[:, bsl, :], in_=ps[:])
                    # Carry relay: o rows 0-1 -> next rhs rows 0-1, on the
                    # otherwise-idle gpsimd engine (SBUF->SBUF fp16; a
                    # [2, 1024] ACT/DVE op costs as much as [128, 1024],
                    # so keeping these off scalar/vector halves their
                    # per-round load).
                    if not tail:
                        nc.gpsimd.tensor_copy(
                            out=xt[i + 1][0:2, bsl, :], in_=o[0:2, bsl, :]
                        )
                oprev = o
            # drain: the tail round's write
            nc.sync.dma_start(
                out=y_d[L * NFULL:T, :, :],
                in_=oprev[:],
            )
    nc.compile()
    return nc


def _get_program():
    if "nc" not in _cache:
        _cache["nc"] = _build_program()
    return _cache["nc"]


def _ensure_axon_hooks_shim():
    """concourse's trace path does `from antenv.axon_hooks import ...`;
    some images lack that module. Install a no-op shim so an externally
    set BASS_TRACE can't crash the run (tracing then degrades to off)."""
    import types

    try:
        import antenv.axon_hooks  # noqa: F401
        return
    except ImportError:
        pass
    try:
        import antenv
    except ImportError:
        return
    mod = types.ModuleType("antenv.axon_hooks")
    mod.get_axon_ntff_profile_hook = lambda: None
    mod.set_axon_ntff_profile_hook = lambda h: None
    mod._kernel_shim = True
    sys.modules["antenv.axon_hooks"] = mod
    antenv.axon_hooks = mod


def _run(x, alpha, beta, trace=False):
    _ensure_axon_hooks_shim()
    from concourse.bass_utils import run_bass_kernel_spmd

    x = np.asarray(x)
    G0, G1, Gt = _build_mats(alpha, beta)
    nc = _get_program()
    in_maps = [
        {
            # [b, t, c] -> [t, b, c] fp16 (see the dram layout note above)
            "x": np.ascontiguousarray(
                x[c * BPC:(c + 1) * BPC]
                .astype(np.float16)
                .transpose(1, 0, 2)
            ),
            "g0": G0, "g1": G1, "gt": Gt,
        }
        for c in range(NCORES)
    ]
    res = run_bass_kernel_spmd(nc, in_maps, list(range(NCORES)), trace=trace)
    out = np.concatenate(
        [res.results[c]["y"].transpose(1, 0, 2) for c in range(NCORES)],
        axis=0,
    ).astype(np.float32)
    return out, res


def kernel(**inputs):
    alpha = float(np.asarray(inputs["alpha"]))
    beta = float(np.asarray(inputs["beta"]))
    out, _ = _run(inputs["x"], alpha, beta, trace=False)
    return out


# revision 19
# speedup vs baseline: 2.3231x; 2.3231x over previous
"""DEMA (double exponential smoothing) Trainium2 kernel — fp16 I/O.

x: [64, 2048, 512] fp32; recurrence over T=2048 is a 2x2 linear
time-invariant system per (batch, channel) lane:

    z_t = A z_{t-1} + B x_t,   y_t = e1^T z_t
    A = [[1-a, 1-a], [-ab, 1-ab]],  B = [a, ab]^T

Blocked scan: chunks of L=126 timesteps. One [128x128] @ [128x512]
matmul per (batch, chunk): rhs rows 0-1 carry the (s, b) state into
the chunk, rows 2..127 carry the chunk's inputs; lhsT columns 0-1
produce the chunk-end state (fed into the next chunk's rhs rows 0-1
via a tiny PSUM->SBUF copy), columns 2..127 produce the outputs.
Batch dim is sharded 8 ways across cores (8 batches per core).

The kernel is HBM-bandwidth bound (~358 GB/s per core). The rel-err
budget (2e-2) dwarfs fp16 quantization (~5e-4 measured end-to-end),
so all HBM traffic is fp16: the host casts x to fp16 per shard, the
kernel computes fp16 matmuls with fp32 PSUM accumulation, writes the
output in fp16, and the host upcasts. That halves traffic vs fp32 to
33 MB/core (~92 us roofline).

DMA plan: ONE read and ONE write dma_start per round, each moving all
8 batches (~1 MB) via a 3D access pattern ([t, b, c]; contiguous 1 KB
runs >= the 512 B line-rate threshold). Reads ride the SP HWDGE ring,
writes the ACT ring — separate FIFOs, so a draining write never
head-of-line-blocks a read; the 16 SDMA engines round-robin between
the two rings at packet granularity, sharing HBM bandwidth.
"""

import sys

import numpy as np

if "/opt/trn_rl_repo" not in sys.path:
    sys.path.insert(0, "/opt/trn_rl_repo")

B, T, C = 64, 2048, 512
NCORES = 8
BPC = B // NCORES  # batches per core
L = 126            # timesteps per full chunk (126 outputs + 2 state rows = 128)
NFULL = 16         # full chunks cover t = 0..2015
LT = T - NFULL * L  # tail chunk, 32 timesteps

NG = 4             # batch groups per round (PSUM granularity)
GB = BPC // NG     # batches per group (2) -> one PSUM tile is [128, GB, 512]

_cache = {}


def _build_mats(alpha, beta):
    """Per-call host precompute of the chunk transfer matrices (float64)."""
    a = np.float64(alpha)
    b = np.float64(beta)
    A = np.array([[1 - a, 1 - a], [-a * b, 1 - a * b]], dtype=np.float64)
    Bv = np.array([a, a * b], dtype=np.float64)
    Ap = [np.eye(2)]
    for _ in range(L):
        Ap.append(Ap[-1] @ A)
    AB = np.stack([Ap[j] @ Bv for j in range(L)])  # [L, 2], A^j B
    w = AB[:, 0]                                   # w_j = e1^T A^j B

    # Generic chunk starting at t0, carry z_{t0-1} in rhs rows 0-1:
    #   z_{t0+tau} = A^{tau+1} z_{t0-1} + sum_k A^{tau-k} B x_{t0+k}
    G1 = np.zeros((128, 128))
    for tau in range(L):
        m = 2 + tau
        G1[0, m] = Ap[tau + 1][0, 0]
        G1[1, m] = Ap[tau + 1][0, 1]
        for k in range(tau + 1):
            G1[2 + k, m] = w[tau - k]
    for j in range(2):
        for jp in range(2):
            G1[j, jp] = Ap[L][jp, j]
    for k in range(L):
        G1[2 + k, 0] = AB[L - 1 - k][0]
        G1[2 + k, 1] = AB[L - 1 - k][1]

    # Chunk 0: z_0 = (x_0, x_1 - x_0), y_0 = x_0, rhs rows 0-1 are zero
    # (and dropped: G0 is [126, 128], round 0's rhs is pure inputs).
    G0 = np.zeros((128, 128))
    G0[2, 2] = 1.0
    for tau in range(1, L):
        m = 2 + tau
        G0[2, m] = Ap[tau][0, 0] - Ap[tau][0, 1]
        G0[3, m] = Ap[tau][0, 1] + w[tau - 1]
        for k in range(2, tau + 1):
            G0[2 + k, m] = w[tau - k]
    for jp in range(2):
        G0[2, jp] = Ap[L - 1][jp, 0] - Ap[L - 1][jp, 1]
        G0[3, jp] = Ap[L - 1][jp, 1] + AB[L - 2][jp]
        for k in range(2, L):
            G0[2 + k, jp] = AB[L - 1 - k][jp]

    # Tail chunk: LT outputs, no state columns.
    Gt = np.zeros((2 + LT, LT))
    for tau in range(LT):
        Gt[0, tau] = Ap[tau + 1][0, 0]
        Gt[1, tau] = Ap[tau + 1][0, 1]
        for k in range(tau + 1):
            Gt[2 + k, tau] = w[tau - k]
    return (
        G0[2:128].astype(np.float16),
        G1.astype(np.float16),
        Gt.astype(np.float16),
    )


def _build_program():
    import concourse.mybir as mybir
    import concourse.tile as tile
    from concourse import bacc

    FP16 = mybir.dt.float16
    FP32 = mybir.dt.float32
    nc = bacc.Bacc(
        "TRN2", target_bir_lowering=False, debug=False, enable_asserts=False
    )
    # x/y live in DRAM pre-permuted to [t, b, c] (host does the transpose):
    # each round's read/write is then one contiguous ~1 MB slab -> 126
    # descriptors of 8 KB instead of 1008 of 1 KB (HWDGE DIRECT2D issue
    # cost and SDMA per-descriptor overhead both drop ~8x).
    x_d = nc.dram_tensor("x", [T, BPC, C], FP16, kind="ExternalInput").ap()
    g0_d = nc.dram_tensor("g0", [L, 128], FP16, kind="ExternalInput").ap()
    g1_d = nc.dram_tensor("g1", [128, 128], FP16, kind="ExternalInput").ap()
    gt_d = nc.dram_tensor("gt", [2 + LT, LT], FP16, kind="ExternalInput").ap()
    y_d = nc.dram_tensor("y", [T, BPC, C], FP16, kind="ExternalOutput").ap()

    with tile.TileContext(nc) as tc:
        with (
            tc.tile_pool(name="g", bufs=1) as gpool,
            tc.tile_pool(name="xp", bufs=4) as xpool,
            tc.tile_pool(name="op", bufs=3) as opool,
            tc.tile_pool(name="ps", bufs=4, space="PSUM") as pspool,
        ):
            g0 = gpool.tile([L, 128], FP16, tag="g0")
            g1 = gpool.tile([128, 128], FP16, tag="g1")
            gt = gpool.tile([2 + LT, LT], FP16, tag="gt")
            nc.scalar.dma_start(out=g0[:], in_=g0_d)
            nc.scalar.dma_start(out=g1[:], in_=g1_d)
            nc.scalar.dma_start(out=gt[:], in_=gt_d)

            def read_round(j):
                """Allocate round j's input tile + issue its read DMA."""
                if j == 0:
                    t = xpool.tile([L, BPC, C], FP16, tag="x")
                    nc.sync.dma_start(out=t[:], in_=x_d[0:L, :, :])
                    return t
                nrows = L if j < NFULL else LT
                t = xpool.tile([2 + nrows, BPC, C], FP16, tag="x")
                nc.sync.dma_start(
                    out=t[2:2 + nrows, :, :],
                    in_=x_d[L * j:L * j + nrows, :, :],
                )
                return t

            # Reads run 2 rounds ahead so a ~6 us DMA completion latency
            # never paces the round loop.
            xt = [read_round(0), read_round(1)]
            oprev = None

            for i in range(NFULL + 1):
                xs = xt[i]
                if i + 2 <= NFULL:
                    xt.append(read_round(i + 2))
                # round i-1's write: issued on the sync ring right after
                # the prefetch so neither DIRECT2D sits in the scalar/
                # vector cast chain; o(i-1) is complete, so no sem stall.
                if i >= 1:
                    nc.sync.dma_start(
                        out=y_d[L * (i - 1):L * i, :, :],
                        in_=oprev[2:, :, :],
                    )
                tail = i == NFULL
                orows = LT if tail else 128
                o = opool.tile([orows, BPC, C], FP16, tag="o")
                for g in range(NG):
                    bsl = slice(g * GB, (g + 1) * GB)
                    ps = pspool.tile([orows, GB, C], FP32, tag="ps")
                    for bb in range(GB):
                        b = g * GB + bb
                        if tail:
                            nc.tensor.matmul(
                                ps[:, bb, :], gt[:], xs[0:2 + LT, b, :],
                                start=True, stop=True,
                            )
                        elif i == 0:
                            nc.tensor.matmul(
                                ps[:, bb, :], g0[:], xs[0:L, b, :],
                                start=True, stop=True,
                            )
                        else:
                            nc.tensor.matmul(
                                ps[:, bb, :], g1[:], xs[:, b, :],
                                start=True, stop=True,
                            )
                    # Output copy first: it is this PSUM tile's ONLY
                    # reader, so the buffer frees as soon as it's done
                    # (pspool bufs=4 makes that gate the next round's
                    # matmul for this group). The carry relay then reads
                    # the fp16 states from o (SBUF->SBUF), not from PSUM;
                    # its consumer, mm(i+1, g), runs ~3 us later.
                    # Alternate engines: 2 copies + 2 relays each.
                    if g % 2 == 0:
                        nc.scalar.copy(out=o[:, bsl, :], in_=ps[:])
                        if not tail:
                            nc.scalar.copy(
                                out=xt[i + 1][0:2, bsl, :], in_=o[0:2, bsl, :]
                            )
                    else:
                        nc.vector.tensor_copy(out=o[:, bsl, :], in_=ps[:])
                        if not tail:
                            nc.vector.tensor_copy(
                                out=xt[i + 1][0:2, bsl, :], in_=o[0:2, bsl, :]
                            )
                oprev = o
            # drain: the tail round's write
            nc.sync.dma_start(
                out=y_d[L * NFULL:T, :, :],
                in_=oprev[:],
            )
    nc.compile()
    return nc


def _get_program():
    if "nc" not in _cache:
        _cache["nc"] = _build_program()
    return _cache["nc"]


def _ensure_axon_hooks_shim():
    """concourse's trace path does `from antenv.axon_hooks import ...`;
    some images lack that module. Install a no-op shim so an externally
    set BASS_TRACE can't crash the run (tracing then degrades to off)."""
    import types

    try:
        import antenv.axon_hooks  # noqa: F401
        return
    except ImportError:
        pass
    try:
        import antenv
    except ImportError:
        return
    mod = types.ModuleType("antenv.axon_hooks")
    mod.get_axon_ntff_profile_hook = lambda: None
    mod.set_axon_ntff_profile_hook = lambda h: None
    mod._kernel_shim = True
    sys.modules["antenv.axon_hooks"] = mod
    antenv.axon_hooks = mod


def _run(x, alpha, beta, trace=False):
    _ensure_axon_hooks_shim()
    from concourse.bass_utils import run_bass_kernel_spmd

    x = np.asarray(x)
    G0, G1, Gt = _build_mats(alpha, beta)
    nc = _get_program()
    in_maps = [
        {
            # [b, t, c] -> [t, b, c] fp16 (see the dram layout note above)
            "x": np.ascontiguousarray(
                x[c * BPC:(c + 1) * BPC]
                .astype(np.float16)
                .transpose(1, 0, 2)
            ),
            "g0": G0, "g1": G1, "gt": Gt,
        }
        for c in range(NCORES)
    ]
    res = run_bass_kernel_spmd(nc, in_maps, list(range(NCORES)), trace=trace)
    out = np.concatenate(
        [res.results[c]["y"].transpose(1, 0, 2) for c in range(NCORES)],
        axis=0,
    ).astype(np.float32)
    return out, res


def kernel(**inputs):
    alpha = float(np.asarray(inputs["alpha"]))
    beta = float(np.asarray(inputs["beta"]))
    out, _ = _run(inputs["x"], alpha, beta, trace=False)
    return out
